# revision 10
# baseline (speedup 1.0000x reference)
"""Distributed Trainium2 (Bass/Tile) kernel for a Qwen3-style attention layer.

Full layer: QKV proj -> per-head RMSNorm (q,k) -> RoPE -> GQA SDPA -> o_proj.

Sharding over 8 NeuronCores:
  - tensor-parallel across heads for QKV+attention: core c owns q-heads
    [4c, 4c+4) and kv-head c; hidden_states replicated.
  - AllToAll exchanges attention context so each core ends with all 4096
    context dims for a 256-token slice; o_proj is then token-parallel with
    Wo replicated (streamed). Output: per-core [256, 4096] chunks that the
    host concatenates. No all-reduce needed.

Compute layout: everything lives transposed ([dim, token]) so the PE array
contracts over the partition axis with N=512 moving tiles in bf16.
"""

import numpy as np
import ml_dtypes

import concourse.bass as bass
import concourse.mybir as mybir
from concourse import bacc
from concourse.tile import TileContext
from concourse.bass_utils import run_bass_kernel_spmd
from concourse.masks import make_identity

F32 = mybir.dt.float32
BF16 = mybir.dt.bfloat16
BF16_NP = ml_dtypes.bfloat16

N_CORES = 8

FULL_CFG = dict(B=2, S=1024, HID=4096, H=32, KV=8, D=128, eps=1e-6)


def build_program(B=2, S=1024, HID=4096, H=32, KV=8, D=128, eps=1e-6):
    cores = N_CORES
    assert D == 128 and H % cores == 0 and KV == cores
    HQ = H // cores            # q heads per core
    T = B * S                  # total tokens
    HCH = HID // 128           # hidden-dim chunks of 128
    TT = min(512, S)           # projection token tile (within batch)
    TPB = S // TT              # projection tiles per batch
    KB = S // 128              # key blocks per batch
    QT = min(512, S)           # attention q tile
    QTB = S // QT              # q tiles per batch
    TC = T // cores            # output tokens per core
    TCB = TC // B              # per-batch token slice per core
    ICH = (H * D) // 128       # o_proj contraction chunks (32)
    OH = 512 if HID >= 1024 else HID // 2   # o_proj hid tile
    HG = HID // (2 * OH)       # o_proj hid groups (2 tiles each)
    scale = float(D) ** -0.5
    MULT = mybir.AluOpType.mult

    nc = bacc.Bacc("TRN2", target_bir_lowering=False, debug=False,
                   num_devices=cores)

    hT = nc.dram_tensor("hT", [B, HCH, 128, S], BF16, kind="ExternalInput")
    wq = nc.dram_tensor("wq", [HQ, 128, HCH * 128], BF16, kind="ExternalInput")
    wk = nc.dram_tensor("wk", [128, HCH * 128], BF16, kind="ExternalInput")
    wv = nc.dram_tensor("wv", [128, HCH * 128], BF16, kind="ExternalInput")
    wo = nc.dram_tensor("wo", [ICH, 128, HID], BF16, kind="ExternalInput")
    cosT = nc.dram_tensor("cosT", [128, S], F32, kind="ExternalInput")
    csinT = nc.dram_tensor("csinT", [128, S], F32, kind="ExternalInput")
    qw = nc.dram_tensor("qw", [128, 1], F32, kind="ExternalInput")
    kw = nc.dram_tensor("kw", [128, 1], F32, kind="ExternalInput")
    out = nc.dram_tensor("out", [TC, HID], F32, kind="ExternalOutput")

    with TileContext(nc) as tc:
        with (
            tc.tile_pool(name="const", bufs=1) as cp,
            tc.tile_pool(name="dram", bufs=1, space="DRAM") as dramp,
        ):
            ones_s = cp.tile([128, 128], BF16)
            nc.vector.memset(ones_s[:, :], 1.0)
            ident = cp.tile([128, 128], BF16)
            make_identity(nc, ident[:, :])
            eps_s = cp.tile([128, 1], F32)
            nc.vector.memset(eps_s[:, :], eps)
            cos_s = cp.tile([128, S], F32)
            nc.sync.dma_start(out=cos_s[:, :], in_=cosT[:, :])
            csin_s = cp.tile([128, S], F32)
            nc.sync.dma_start(out=csin_s[:, :], in_=csinT[:, :])
            qw_s = cp.tile([128, 1], F32)
            nc.sync.dma_start(out=qw_s[:, :], in_=qw[:, :])
            kw_s = cp.tile([128, 1], F32)
            nc.sync.dma_start(out=kw_s[:, :], in_=kw[:, :])

            a2a_in = [dramp.tile([H * D, TCB], BF16, tag=f"a2ai{b}", name=f"a2ai{b}")
                      for b in range(B)]
            a2a_out = [dramp.tile([H * D, TCB], BF16, tag=f"a2ao{b}", name=f"a2ao{b}")
                       for b in range(B)]

            with (
                tc.tile_pool(name="hid", bufs=HCH) as p_hid,
                tc.tile_pool(name="wts", bufs=2) as p_w,
                tc.tile_pool(name="qkv", bufs=1) as p_qkv,
                tc.tile_pool(name="work", bufs=2) as p_work,
                tc.tile_pool(name="pt", bufs=3) as p_pt,
                tc.tile_pool(name="psmm", bufs=3, space="PSUM") as ps_mm,
                tc.tile_pool(name="psred", bufs=2, space="PSUM") as ps_red,
                tc.tile_pool(name="psctx", bufs=2, space="PSUM") as ps_ctx,
                tc.tile_pool(name="psvtr", bufs=1, space="PSUM") as ps_vtr,
            ):
                qT_s = p_qkv.tile([128, HQ * T], BF16, tag="qT")
                kT_s = p_qkv.tile([128, T], BF16, tag="kT")
                vnat_s = p_qkv.tile([128, T], BF16, tag="vnat")
                ctxT_s = p_qkv.tile([128, HQ * T], BF16, tag="ctxT")

                for b in range(B):
                    # ---- hidden chunks for this batch into SBUF ----
                    sc_ = nc.enter_named_scope(f"proj{b}", True)[0]
                    w0_t = p_w.tile([128, HCH * 128], BF16, tag="w",
                                    name="w0")
                    nc.sync.dma_start(out=w0_t[:, :], in_=wq[0])
                    hch = []
                    for ch in range(HCH):
                        t_ = p_hid.tile([128, S], BF16, tag="hid", name="hid")
                        nc.sync.dma_start(out=t_[:, :], in_=hT[b, ch, :, :])
                        hch.append(t_)

                    # ---- QKV projections (+norm+rope for q,k) ----
                    for ob in range(HQ + 2):
                        if ob == 0:
                            w_t = w0_t
                        else:
                            w_t = p_w.tile([128, HCH * 128], BF16, tag="w",
                                           name="w")
                            src = (wq[ob] if ob < HQ else
                                   (wk[:, :] if ob == HQ else wv[:, :]))
                            nc.sync.dma_start(out=w_t[:, :], in_=src)
                        for tt in range(TPB):
                            ps = ps_mm.tile([128, TT], F32, tag="mm")
                            for ch in range(HCH):
                                nc.tensor.matmul(
                                    ps[:, :],
                                    lhsT=w_t[:, ch * 128:(ch + 1) * 128],
                                    rhs=hch[ch][:, tt * TT:(tt + 1) * TT],
                                    start=(ch == 0), stop=(ch == HCH - 1))
                            tg = b * S + tt * TT
                            pos = tt * TT
                            if ob <= HQ:
                                # per-head rmsnorm + rope
                                is_q = ob < HQ
                                dst = (qT_s[:, ob * T + tg: ob * T + tg + TT]
                                       if is_q else kT_s[:, tg: tg + TT])
                                wcol = qw_s if is_q else kw_s
                                sq = p_work.tile([128, TT], BF16, tag="sq")
                                nc.scalar.square(sq[:, :], ps[:, :])
                                ssq = ps_red.tile([128, TT], F32, tag="red")
                                nc.tensor.matmul(ssq[:, :], lhsT=ones_s[:, :],
                                                 rhs=sq[:, :], start=True,
                                                 stop=True)
                                std = p_work.tile([128, TT], F32, tag="std")
                                nc.scalar.activation(
                                    std[:, :], ssq[:, :],
                                    mybir.ActivationFunctionType.Sqrt,
                                    bias=eps_s[:, :], scale=1.0 / D)
                                rs = p_work.tile([128, TT], F32, tag="rs")
                                nc.vector.reciprocal_approx_fast(rs[:, :], std[:, :])
                                qn = p_work.tile([128, TT], F32, tag="qn")
                                nc.vector.scalar_tensor_tensor(
                                    qn[:, :], in0=ps[:, :], scalar=wcol[:, :],
                                    in1=rs[:, :], op0=MULT, op1=MULT)
                                qsw = p_work.tile([128, TT], F32, tag="qsw")
                                nc.sync.dma_start(out=qsw[0:64, :],
                                                  in_=qn[64:128, :])
                                nc.sync.dma_start(out=qsw[64:128, :],
                                                  in_=qn[0:64, :])
                                t1 = p_work.tile([128, TT], F32, tag="t1")
                                nc.vector.tensor_mul(
                                    t1[:, :], qn[:, :],
                                    cos_s[:, pos: pos + TT])
                                t2 = p_work.tile([128, TT], F32, tag="t2")
                                nc.vector.tensor_mul(
                                    t2[:, :], qsw[:, :],
                                    csin_s[:, pos: pos + TT])
                                nc.vector.tensor_add(dst, t1[:, :], t2[:, :])
                            else:
                                # v: cast to bf16, transpose to [t, d] blocks
                                vt = p_work.tile([128, TT], BF16, tag="vt")
                                nc.scalar.copy(vt[:, :], ps[:, :])
                                for tb in range(TT // 128):
                                    vtr = ps_vtr.tile([128, 128], BF16,
                                                      tag="vtr")
                                    nc.tensor.transpose(
                                        vtr[:, :], vt[:, tb * 128:(tb + 1) * 128],
                                        ident[:, :])
                                    tbg = tg // 128 + tb
                                    nc.scalar.copy(
                                        vnat_s[:, tbg * 128:(tbg + 1) * 128],
                                        vtr[:, :])

                    # ---- attention for this batch ----
                    nc.leave_named_scope(f"proj{b}", sc_, True)
                    sc_ = nc.enter_named_scope(f"attn{b}", True)[0]
                    for h in range(HQ):
                        pts = []
                        for qt in range(QTB):
                            pt_t = p_pt.tile([128, KB * QT], BF16, tag="pT", name="pT")
                            qoff = h * T + b * S + qt * QT
                            for kb in range(KB):
                                sps = ps_mm.tile([128, QT], F32, tag="mm")
                                nc.tensor.matmul(
                                    sps[:, :],
                                    lhsT=kT_s[:, b * S + kb * 128: b * S + (kb + 1) * 128],
                                    rhs=qT_s[:, qoff: qoff + QT],
                                    start=True, stop=True)
                                nc.scalar.activation(
                                    pt_t[:, kb * QT:(kb + 1) * QT], sps[:, :],
                                    mybir.ActivationFunctionType.Exp,
                                    scale=scale)
                            pts.append(pt_t)
                        ctxs = [ps_ctx.tile([128, QT], F32, tag="ctx", name="ctx")
                                for _ in range(QTB)]
                        for kb in range(KB):
                            tbg = (b * S) // 128 + kb
                            for qt in range(QTB):
                                nc.tensor.matmul(
                                    ctxs[qt][:, :],
                                    lhsT=vnat_s[:, tbg * 128:(tbg + 1) * 128],
                                    rhs=pts[qt][:, kb * QT:(kb + 1) * QT],
                                    start=(kb == 0), stop=(kb == KB - 1))
                        for qt in range(QTB):
                            denp = p_work.tile([128, QT], F32, tag="denp")
                            nc.gpsimd.tensor_add(denp[:, :],
                                                 pts[qt][:, 0:QT],
                                                 pts[qt][:, QT:2 * QT])
                            for kb in range(2, KB):
                                nc.gpsimd.tensor_add(
                                    denp[:, :], denp[:, :],
                                    pts[qt][:, kb * QT:(kb + 1) * QT])
                            denb = p_work.tile([128, QT], BF16, tag="denb")
                            nc.scalar.copy(denb[:, :], denp[:, :])
                            dps = ps_red.tile([128, QT], F32, tag="red",
                                              name="dps")
                            nc.tensor.matmul(dps[:, :], lhsT=ones_s[:, :],
                                             rhs=denb[:, :], start=True,
                                             stop=True)
                            rec = p_work.tile([128, QT], F32, tag="rec")
                            nc.vector.reciprocal_approx_fast(rec[:, :],
                                                             dps[:, :])
                            qoff = h * T + b * S + qt * QT
                            nc.vector.tensor_mul(
                                ctxT_s[:, qoff: qoff + QT], ctxs[qt][:, :],
                                rec[:, :])

                    # ---- ship context: AllToAll for this batch ----
                    nc.leave_named_scope(f"attn{b}", sc_, True)
                    sc_ = nc.enter_named_scope(f"a2a{b}", True)[0]
                    ctx_v = ctxT_s[:, :].rearrange("p (h t) -> p h t", h=HQ)
                    for j in range(cores):
                        nc.sync.dma_start(
                            out=a2a_in[b][j * HQ * 128:(j + 1) * HQ * 128, :]
                            .rearrange("(h p) t -> p h t", h=HQ),
                            in_=ctx_v[:, :, b * S + j * TCB:
                                      b * S + (j + 1) * TCB])
                    nc.gpsimd.collective_compute(
                        "AllToAll", mybir.AluOpType.bypass,
                        replica_groups=[list(range(cores))],
                        ins=[a2a_in[b].opt()],
                        outs=[a2a_out[b].opt()])
                    nc.leave_named_scope(f"a2a{b}", sc_, True)

            # ---- o_proj: token-parallel with streamed full Wo ----
            with (
                tc.tile_pool(name="cx", bufs=1) as p_cx,
                tc.tile_pool(name="wo", bufs=6) as p_wo,
                tc.tile_pool(name="oo", bufs=4) as p_oo,
                tc.tile_pool(name="ps3", bufs=8, space="PSUM") as ps3,
            ):
                sc_ = nc.enter_named_scope("oproj", True)[0]
                cx_s = []
                for b in range(B):
                    t_ = p_cx.tile([128, ICH * TCB], BF16, tag=f"cx{b}", name=f"cx{b}")
                    nc.sync.dma_start(
                        out=t_[:, :].rearrange("p (ic t) -> p ic t", ic=ICH),
                        in_=a2a_out[b][:, :].rearrange("(ic p) t -> p ic t",
                                                       ic=ICH))
                    cx_s.append(t_)
                for hg in range(HG):
                    wts = []
                    for ic in range(ICH):
                        wo_t = p_wo.tile([128, 2 * OH], BF16, tag="wo",
                                         name="wo", bufs=ICH + 2)
                        nc.sync.dma_start(
                            out=wo_t[:, :],
                            in_=wo[ic, :, hg * 2 * OH:(hg + 1) * 2 * OH])
                        wts.append(wo_t)
                    for b in range(B):
                        pso = [ps3.tile([TCB, OH], F32, tag="o", name="pso")
                               for _ in range(2)]
                        for ic in range(ICH):
                            for ht in range(2):
                                nc.tensor.matmul(
                                    pso[ht][:, :],
                                    lhsT=cx_s[b][:, ic * TCB:(ic + 1) * TCB],
                                    rhs=wts[ic][:, ht * OH:(ht + 1) * OH],
                                    start=(ic == 0), stop=(ic == ICH - 1))
                        for ht in range(2):
                            ot = p_oo.tile([TCB, OH], F32, tag="oout",
                                           name="oout")
                            nc.vector.tensor_copy(ot[:, :], pso[ht][:, :])
                            nc.sync.dma_start(
                                out=out[b * TCB:(b + 1) * TCB,
                                        hg * 2 * OH + ht * OH:
                                        hg * 2 * OH + (ht + 1) * OH],
                                in_=ot[:, :])
                nc.leave_named_scope("oproj", sc_, True)

    nc.compile()
    return nc


def host_prep(inputs, B=2, S=1024, HID=4096, H=32, KV=8, D=128, eps=1e-6):
    """Shard + lay out the full inputs into per-core in_maps."""
    cores = N_CORES
    HQ = H // cores
    T = B * S
    HCH = HID // 128
    ICH = (H * D) // 128

    hs = np.ascontiguousarray(inputs["hidden_states"], dtype=np.float32)
    fc = np.asarray(inputs["freqs_cis"], dtype=np.float32)
    Wq = np.asarray(inputs["Wq"], dtype=np.float32)
    Wk = np.asarray(inputs["Wk"], dtype=np.float32)
    Wv = np.asarray(inputs["Wv"], dtype=np.float32)
    Wo = np.asarray(inputs["Wo"], dtype=np.float32)
    qnw = np.asarray(inputs["q_norm_w"], dtype=np.float32)
    knw = np.asarray(inputs["k_norm_w"], dtype=np.float32)

    # hidden^T chunks: hT[b, ch, p, s] = hs[b, s, ch*128+p]
    hT = np.ascontiguousarray(
        hs.transpose(0, 2, 1).reshape(B, HCH, 128, S)).astype(BF16_NP)

    cos, sin, nsin = fc[0], fc[1], fc[2]      # [S, D]
    cosT = np.ascontiguousarray(cos.T)        # [128, S]
    csinT = np.concatenate([nsin.T[0:64], sin.T[64:128]], axis=0)
    csinT = np.ascontiguousarray(csinT)
    qw_col = np.ascontiguousarray(qnw.reshape(128, 1))
    kw_col = np.ascontiguousarray(knw.reshape(128, 1))

    # Wo^T chunks: wo[ic, p, hid] = Wo[hid, ic*128+p]
    woT = np.ascontiguousarray(Wo.T.reshape(ICH, 128, HID)).astype(BF16_NP)

    def prep_w(Wm, nblocks):
        # [nblocks, p, ch*128] with w[ob, p, ch*128+j] = Wm[ob*128+j, ch*128+p]
        a = Wm.reshape(nblocks, 128, HCH, 128).transpose(0, 3, 2, 1)
        return np.ascontiguousarray(a.reshape(nblocks, 128, HCH * 128)) \
            .astype(BF16_NP)

    in_maps = []
    for c in range(cores):
        Wq_c = Wq[c * HQ * D:(c + 1) * HQ * D]
        Wk_c = Wk[c * D:(c + 1) * D]
        Wv_c = Wv[c * D:(c + 1) * D]
        in_maps.append({
            "hT": hT,
            "wq": prep_w(Wq_c, HQ),
            "wk": prep_w(Wk_c, 1)[0],
            "wv": prep_w(Wv_c, 1)[0],
            "wo": woT,
            "cosT": cosT,
            "csinT": csinT,
            "qw": qw_col,
            "kw": kw_col,
        })
    return in_maps


def gather_output(results, B=2, S=1024, HID=4096, **_):
    cores = N_CORES
    TCB = (B * S) // cores // B
    out = np.empty((B, S, HID), dtype=np.float32)
    for c in range(cores):
        o = results[c]["out"]
        for b in range(B):
            out[b, c * TCB:(c + 1) * TCB] = o[b * TCB:(b + 1) * TCB]
    return out


_NC_CACHE = {}


def kernel(**inputs) -> np.ndarray:
    cfg = FULL_CFG
    key = tuple(sorted(cfg.items()))
    if key not in _NC_CACHE:
        _NC_CACHE[key] = build_program(**cfg)
    nc = _NC_CACHE[key]
    in_maps = host_prep(inputs, **cfg)
    res = run_bass_kernel_spmd(nc, in_maps, core_ids=list(range(N_CORES)))
    return gather_output(res.results, **cfg)


# revision 12
# speedup vs baseline: 1.1064x; 1.1064x over previous
"""Distributed Trainium2 (Bass/Tile) kernel for a Qwen3-style attention layer.

Full layer: QKV proj -> per-head RMSNorm (q,k) -> RoPE -> GQA SDPA -> o_proj.

Sharding over 8 NeuronCores:
  - tensor-parallel across heads for QKV+attention: core c owns q-heads
    [4c, 4c+4) and kv-head c; hidden_states replicated.
  - AllToAll exchanges attention context so each core ends with all 4096
    context dims for a 256-token slice; o_proj is then token-parallel with
    Wo replicated (streamed). Output: per-core [256, 4096] chunks that the
    host concatenates. No all-reduce needed.

Compute layout: everything lives transposed ([dim, token]) so the PE array
contracts over the partition axis with N=512 moving tiles in bf16.
"""

import numpy as np
import ml_dtypes

import concourse.bass as bass
import concourse.mybir as mybir
from concourse import bacc
from concourse.tile import TileContext
from concourse.bass_utils import run_bass_kernel_spmd
from concourse.masks import make_identity

F32 = mybir.dt.float32
BF16 = mybir.dt.bfloat16
BF16_NP = ml_dtypes.bfloat16

N_CORES = 8

FULL_CFG = dict(B=2, S=1024, HID=4096, H=32, KV=8, D=128, eps=1e-6)


def build_program(B=2, S=1024, HID=4096, H=32, KV=8, D=128, eps=1e-6):
    cores = N_CORES
    assert D == 128 and H % cores == 0 and KV == cores
    HQ = H // cores            # q heads per core
    T = B * S                  # total tokens
    HCH = HID // 128           # hidden-dim chunks of 128
    TT = min(512, S)           # projection token tile (within batch)
    TPB = S // TT              # projection tiles per batch
    KB = S // 128              # key blocks per batch
    QT = min(512, S)           # attention q tile
    QTB = S // QT              # q tiles per batch
    TC = T // cores            # output tokens per core
    TCB = TC // B              # per-batch token slice per core
    ICH = (H * D) // 128       # o_proj contraction chunks (32)
    OH = 512 if HID >= 1024 else HID // 2   # o_proj hid tile
    HG = HID // (2 * OH)       # o_proj hid groups (2 tiles each)
    scale = float(D) ** -0.5
    MULT = mybir.AluOpType.mult

    nc = bacc.Bacc("TRN2", target_bir_lowering=False, debug=False,
                   num_devices=cores)

    hT = nc.dram_tensor("hT", [B, HCH, 128, S], BF16, kind="ExternalInput")
    wq = nc.dram_tensor("wq", [HQ, 128, HCH * 128], BF16, kind="ExternalInput")
    wk = nc.dram_tensor("wk", [128, HCH * 128], BF16, kind="ExternalInput")
    wv = nc.dram_tensor("wv", [128, HCH * 128], BF16, kind="ExternalInput")
    wo = nc.dram_tensor("wo", [ICH, 128, HID], BF16, kind="ExternalInput")
    cosT = nc.dram_tensor("cosT", [128, S], BF16, kind="ExternalInput")
    csinT = nc.dram_tensor("csinT", [128, S], BF16, kind="ExternalInput")
    qw = nc.dram_tensor("qw", [128, 1], F32, kind="ExternalInput")
    kw = nc.dram_tensor("kw", [128, 1], F32, kind="ExternalInput")
    out = nc.dram_tensor("out", [TC, HID], F32, kind="ExternalOutput")

    with TileContext(nc) as tc:
        with (
            tc.tile_pool(name="const", bufs=1) as cp,
            tc.tile_pool(name="dram", bufs=1, space="DRAM") as dramp,
        ):
            ones_s = cp.tile([128, 128], BF16)
            nc.vector.memset(ones_s[:, :], 1.0)
            ident = cp.tile([128, 128], BF16)
            make_identity(nc, ident[:, :])
            eps_s = cp.tile([128, 1], F32)
            nc.vector.memset(eps_s[:, :], eps)
            cos_s = cp.tile([128, S], BF16)
            nc.sync.dma_start(out=cos_s[:, :], in_=cosT[:, :])
            csin_s = cp.tile([128, S], BF16)
            nc.sync.dma_start(out=csin_s[:, :], in_=csinT[:, :])
            qw_s = cp.tile([128, 1], F32)
            nc.sync.dma_start(out=qw_s[:, :], in_=qw[:, :])
            kw_s = cp.tile([128, 1], F32)
            nc.sync.dma_start(out=kw_s[:, :], in_=kw[:, :])

            a2a_in = [dramp.tile([H * D, TCB], BF16, tag=f"a2ai{b}", name=f"a2ai{b}")
                      for b in range(B)]
            a2a_out = [dramp.tile([H * D, TCB], BF16, tag=f"a2ao{b}", name=f"a2ao{b}")
                       for b in range(B)]

            with (
                tc.tile_pool(name="hid", bufs=HCH) as p_hid,
                tc.tile_pool(name="wts", bufs=2) as p_w,
                tc.tile_pool(name="qkv", bufs=1) as p_qkv,
                tc.tile_pool(name="work", bufs=2) as p_work,
                tc.tile_pool(name="pt", bufs=4) as p_pt,
                tc.tile_pool(name="psmm", bufs=3, space="PSUM") as ps_mm,
                tc.tile_pool(name="psred", bufs=2, space="PSUM") as ps_red,
                tc.tile_pool(name="psctx", bufs=2, space="PSUM") as ps_ctx,
                tc.tile_pool(name="psvtr", bufs=1, space="PSUM") as ps_vtr,
            ):
                qT_s = p_qkv.tile([128, HQ * T], BF16, tag="qT")
                kT_s = p_qkv.tile([128, T], BF16, tag="kT")
                vnat_s = p_qkv.tile([128, T], BF16, tag="vnat")
                ctxT_s = p_qkv.tile([128, HQ * T], BF16, tag="ctxT")

                for b in range(B):
                    # ---- hidden chunks for this batch into SBUF ----
                    sc_ = nc.enter_named_scope(f"proj{b}", True)[0]
                    w0_t = p_w.tile([128, HCH * 128], BF16, tag="w",
                                    name="w0")
                    nc.sync.dma_start(out=w0_t[:, :], in_=wq[0])
                    hch = []
                    for ch in range(HCH):
                        t_ = p_hid.tile([128, S], BF16, tag="hid", name="hid")
                        nc.sync.dma_start(out=t_[:, :], in_=hT[b, ch, :, :])
                        hch.append(t_)

                    # ---- QKV projections (+norm+rope for q,k) ----
                    for ob in range(HQ + 2):
                        if ob == 0:
                            w_t = w0_t
                        else:
                            w_t = p_w.tile([128, HCH * 128], BF16, tag="w",
                                           name="w")
                            src = (wq[ob] if ob < HQ else
                                   (wk[:, :] if ob == HQ else wv[:, :]))
                            nc.sync.dma_start(out=w_t[:, :], in_=src)
                        for tt in range(TPB):
                            ps = ps_mm.tile([128, TT], F32, tag="mm")
                            for ch in range(HCH):
                                nc.tensor.matmul(
                                    ps[:, :],
                                    lhsT=w_t[:, ch * 128:(ch + 1) * 128],
                                    rhs=hch[ch][:, tt * TT:(tt + 1) * TT],
                                    start=(ch == 0), stop=(ch == HCH - 1))
                            tg = b * S + tt * TT
                            pos = tt * TT
                            if ob <= HQ:
                                # per-head rmsnorm + rope
                                is_q = ob < HQ
                                dst = (qT_s[:, ob * T + tg: ob * T + tg + TT]
                                       if is_q else kT_s[:, tg: tg + TT])
                                wcol = qw_s if is_q else kw_s
                                sq = p_work.tile([128, TT], BF16, tag="sq")
                                nc.scalar.square(sq[:, :], ps[:, :])
                                ssq = ps_red.tile([128, TT], F32, tag="red")
                                nc.tensor.matmul(ssq[:, :], lhsT=ones_s[:, :],
                                                 rhs=sq[:, :], start=True,
                                                 stop=True)
                                std = p_work.tile([128, TT], F32, tag="std")
                                nc.scalar.activation(
                                    std[:, :], ssq[:, :],
                                    mybir.ActivationFunctionType.Sqrt,
                                    bias=eps_s[:, :], scale=1.0 / D)
                                rs = p_work.tile([128, TT], F32, tag="rs")
                                nc.vector.reciprocal_approx_fast(rs[:, :], std[:, :])
                                qn = p_work.tile([128, TT], F32, tag="qn")
                                nc.vector.scalar_tensor_tensor(
                                    qn[:, :], in0=ps[:, :], scalar=wcol[:, :],
                                    in1=rs[:, :], op0=MULT, op1=MULT)
                                qsw = p_work.tile([128, TT], F32, tag="qsw")
                                nc.sync.dma_start(out=qsw[0:64, :],
                                                  in_=qn[64:128, :])
                                nc.sync.dma_start(out=qsw[64:128, :],
                                                  in_=qn[0:64, :])
                                t1 = p_work.tile([128, TT], F32, tag="t1")
                                nc.vector.tensor_mul(
                                    t1[:, :], qn[:, :],
                                    cos_s[:, pos: pos + TT])
                                t2 = p_work.tile([128, TT], BF16, tag="t2")
                                nc.vector.tensor_mul(
                                    t2[:, :], qsw[:, :],
                                    csin_s[:, pos: pos + TT])
                                nc.vector.tensor_add(dst, t1[:, :], t2[:, :])
                            else:
                                # v: cast to bf16, transpose to [t, d] blocks
                                vt = p_work.tile([128, TT], BF16, tag="vt")
                                nc.scalar.copy(vt[:, :], ps[:, :])
                                for tb in range(TT // 128):
                                    vtr = ps_vtr.tile([128, 128], BF16,
                                                      tag="vtr")
                                    nc.tensor.transpose(
                                        vtr[:, :], vt[:, tb * 128:(tb + 1) * 128],
                                        ident[:, :])
                                    tbg = tg // 128 + tb
                                    nc.scalar.copy(
                                        vnat_s[:, tbg * 128:(tbg + 1) * 128],
                                        vtr[:, :])

                    # ---- attention for this batch ----
                    nc.leave_named_scope(f"proj{b}", sc_, True)
                    sc_ = nc.enter_named_scope(f"attn{b}", True)[0]
                    for h in range(HQ):
                        pts = []
                        for qt in range(QTB):
                            pt_t = p_pt.tile([128, KB * QT], BF16, tag="pT", name="pT")
                            qoff = h * T + b * S + qt * QT
                            for kb in range(KB):
                                sps = ps_mm.tile([128, QT], F32, tag="mm")
                                nc.tensor.matmul(
                                    sps[:, :],
                                    lhsT=kT_s[:, b * S + kb * 128: b * S + (kb + 1) * 128],
                                    rhs=qT_s[:, qoff: qoff + QT],
                                    start=True, stop=True)
                                nc.scalar.activation(
                                    pt_t[:, kb * QT:(kb + 1) * QT], sps[:, :],
                                    mybir.ActivationFunctionType.Exp,
                                    scale=scale)
                            pts.append(pt_t)
                        ctxs = [ps_ctx.tile([128, QT], F32, tag="ctx", name="ctx")
                                for _ in range(QTB)]
                        for kb in range(KB):
                            tbg = (b * S) // 128 + kb
                            for qt in range(QTB):
                                nc.tensor.matmul(
                                    ctxs[qt][:, :],
                                    lhsT=vnat_s[:, tbg * 128:(tbg + 1) * 128],
                                    rhs=pts[qt][:, kb * QT:(kb + 1) * QT],
                                    start=(kb == 0), stop=(kb == KB - 1))
                        for qt in range(QTB):
                            denp = p_work.tile([128, QT], F32, tag="denp")
                            dent = p_work.tile([128, QT], F32, tag="dent")
                            if KB == 2:
                                nc.vector.tensor_add(denp[:, :],
                                                     pts[qt][:, 0:QT],
                                                     pts[qt][:, QT:2 * QT])
                            else:
                                assert KB % 4 == 0
                                nc.vector.tensor_add(denp[:, :],
                                                     pts[qt][:, 0:QT],
                                                     pts[qt][:, QT:2 * QT])
                                nc.vector.tensor_add(
                                    dent[:, :], pts[qt][:, 2 * QT:3 * QT],
                                    pts[qt][:, 3 * QT:4 * QT])
                                nc.vector.tensor_add(denp[:, :], denp[:, :],
                                                     dent[:, :])
                                for g in range(1, KB // 4):
                                    nc.vector.tensor_add(
                                        dent[:, :],
                                        pts[qt][:, 4 * g * QT:(4 * g + 1) * QT],
                                        pts[qt][:, (4 * g + 1) * QT:(4 * g + 2) * QT])
                                    nc.vector.tensor_add(denp[:, :], denp[:, :],
                                                         dent[:, :])
                                    nc.vector.tensor_add(
                                        dent[:, :],
                                        pts[qt][:, (4 * g + 2) * QT:(4 * g + 3) * QT],
                                        pts[qt][:, (4 * g + 3) * QT:(4 * g + 4) * QT])
                                    nc.vector.tensor_add(denp[:, :], denp[:, :],
                                                         dent[:, :])
                            denb = p_work.tile([128, QT], BF16, tag="denb")
                            nc.scalar.copy(denb[:, :], denp[:, :])
                            dps = ps_red.tile([128, QT], F32, tag="red",
                                              name="dps")
                            nc.tensor.matmul(dps[:, :], lhsT=ones_s[:, :],
                                             rhs=denb[:, :], start=True,
                                             stop=True)
                            rec = p_work.tile([128, QT], F32, tag="rec")
                            nc.vector.reciprocal_approx_fast(rec[:, :],
                                                             dps[:, :])
                            qoff = h * T + b * S + qt * QT
                            nc.vector.tensor_mul(
                                ctxT_s[:, qoff: qoff + QT], ctxs[qt][:, :],
                                rec[:, :])

                    # ---- ship context: AllToAll for this batch ----
                    nc.leave_named_scope(f"attn{b}", sc_, True)
                    sc_ = nc.enter_named_scope(f"a2a{b}", True)[0]
                    ctx_v = ctxT_s[:, :].rearrange("p (h t) -> p h t", h=HQ)
                    for j in range(cores):
                        nc.sync.dma_start(
                            out=a2a_in[b][j * HQ * 128:(j + 1) * HQ * 128, :]
                            .rearrange("(h p) t -> p h t", h=HQ),
                            in_=ctx_v[:, :, b * S + j * TCB:
                                      b * S + (j + 1) * TCB])
                    nc.gpsimd.collective_compute(
                        "AllToAll", mybir.AluOpType.bypass,
                        replica_groups=[list(range(cores))],
                        ins=[a2a_in[b].opt()],
                        outs=[a2a_out[b].opt()])
                    nc.leave_named_scope(f"a2a{b}", sc_, True)

            # ---- o_proj: token-parallel with streamed full Wo ----
            with (
                tc.tile_pool(name="cx", bufs=1) as p_cx,
                tc.tile_pool(name="wo", bufs=6) as p_wo,
                tc.tile_pool(name="oo", bufs=4) as p_oo,
                tc.tile_pool(name="ps3", bufs=8, space="PSUM") as ps3,
            ):
                sc_ = nc.enter_named_scope("oproj", True)[0]
                cx_s = []
                for b in range(B):
                    t_ = p_cx.tile([128, ICH * TCB], BF16, tag=f"cx{b}", name=f"cx{b}")
                    nc.sync.dma_start(
                        out=t_[:, :].rearrange("p (ic t) -> p ic t", ic=ICH),
                        in_=a2a_out[b][:, :].rearrange("(ic p) t -> p ic t",
                                                       ic=ICH))
                    cx_s.append(t_)
                for hg in range(HG):
                    wts = []
                    for ic in range(ICH):
                        wo_t = p_wo.tile([128, 2 * OH], BF16, tag="wo",
                                         name="wo", bufs=ICH + 2)
                        nc.sync.dma_start(
                            out=wo_t[:, :],
                            in_=wo[ic, :, hg * 2 * OH:(hg + 1) * 2 * OH])
                        wts.append(wo_t)
                    for b in range(B):
                        pso = [ps3.tile([TCB, OH], F32, tag="o", name="pso")
                               for _ in range(2)]
                        for ic in range(ICH):
                            for ht in range(2):
                                nc.tensor.matmul(
                                    pso[ht][:, :],
                                    lhsT=cx_s[b][:, ic * TCB:(ic + 1) * TCB],
                                    rhs=wts[ic][:, ht * OH:(ht + 1) * OH],
                                    start=(ic == 0), stop=(ic == ICH - 1))
                        for ht in range(2):
                            ot = p_oo.tile([TCB, OH], F32, tag="oout",
                                           name="oout")
                            nc.vector.tensor_copy(ot[:, :], pso[ht][:, :])
                            nc.sync.dma_start(
                                out=out[b * TCB:(b + 1) * TCB,
                                        hg * 2 * OH + ht * OH:
                                        hg * 2 * OH + (ht + 1) * OH],
                                in_=ot[:, :])
                nc.leave_named_scope("oproj", sc_, True)

    nc.compile()
    return nc


def host_prep(inputs, B=2, S=1024, HID=4096, H=32, KV=8, D=128, eps=1e-6):
    """Shard + lay out the full inputs into per-core in_maps."""
    cores = N_CORES
    HQ = H // cores
    T = B * S
    HCH = HID // 128
    ICH = (H * D) // 128

    hs = np.ascontiguousarray(inputs["hidden_states"], dtype=np.float32)
    fc = np.asarray(inputs["freqs_cis"], dtype=np.float32)
    Wq = np.asarray(inputs["Wq"], dtype=np.float32)
    Wk = np.asarray(inputs["Wk"], dtype=np.float32)
    Wv = np.asarray(inputs["Wv"], dtype=np.float32)
    Wo = np.asarray(inputs["Wo"], dtype=np.float32)
    qnw = np.asarray(inputs["q_norm_w"], dtype=np.float32)
    knw = np.asarray(inputs["k_norm_w"], dtype=np.float32)

    # hidden^T chunks: hT[b, ch, p, s] = hs[b, s, ch*128+p]
    hT = np.ascontiguousarray(
        hs.transpose(0, 2, 1).reshape(B, HCH, 128, S)).astype(BF16_NP)

    cos, sin, nsin = fc[0], fc[1], fc[2]      # [S, D]
    cosT = np.ascontiguousarray(cos.T).astype(BF16_NP)    # [128, S]
    csinT = np.concatenate([nsin.T[0:64], sin.T[64:128]], axis=0)
    csinT = np.ascontiguousarray(csinT).astype(BF16_NP)
    qw_col = np.ascontiguousarray(qnw.reshape(128, 1))
    kw_col = np.ascontiguousarray(knw.reshape(128, 1))

    # Wo^T chunks: wo[ic, p, hid] = Wo[hid, ic*128+p]
    woT = np.ascontiguousarray(Wo.T.reshape(ICH, 128, HID)).astype(BF16_NP)

    def prep_w(Wm, nblocks):
        # [nblocks, p, ch*128] with w[ob, p, ch*128+j] = Wm[ob*128+j, ch*128+p]
        a = Wm.reshape(nblocks, 128, HCH, 128).transpose(0, 3, 2, 1)
        return np.ascontiguousarray(a.reshape(nblocks, 128, HCH * 128)) \
            .astype(BF16_NP)

    in_maps = []
    for c in range(cores):
        Wq_c = Wq[c * HQ * D:(c + 1) * HQ * D]
        Wk_c = Wk[c * D:(c + 1) * D]
        Wv_c = Wv[c * D:(c + 1) * D]
        in_maps.append({
            "hT": hT,
            "wq": prep_w(Wq_c, HQ),
            "wk": prep_w(Wk_c, 1)[0],
            "wv": prep_w(Wv_c, 1)[0],
            "wo": woT,
            "cosT": cosT,
            "csinT": csinT,
            "qw": qw_col,
            "kw": kw_col,
        })
    return in_maps


def gather_output(results, B=2, S=1024, HID=4096, **_):
    cores = N_CORES
    TCB = (B * S) // cores // B
    out = np.empty((B, S, HID), dtype=np.float32)
    for c in range(cores):
        o = results[c]["out"]
        for b in range(B):
            out[b, c * TCB:(c + 1) * TCB] = o[b * TCB:(b + 1) * TCB]
    return out


_NC_CACHE = {}


def kernel(**inputs) -> np.ndarray:
    cfg = FULL_CFG
    key = tuple(sorted(cfg.items()))
    if key not in _NC_CACHE:
        _NC_CACHE[key] = build_program(**cfg)
    nc = _NC_CACHE[key]
    in_maps = host_prep(inputs, **cfg)
    res = run_bass_kernel_spmd(nc, in_maps, core_ids=list(range(N_CORES)))
    return gather_output(res.results, **cfg)
